# revision 1
# baseline (speedup 1.0000x reference)
"""Multi-head cross-attention on 8 Trainium2 NeuronCores.

Problem shapes (hardcoded): B=4, Ld=1024, Le=2048, d_model=1024, 8 heads x 128.
Sharding: core c handles batch b=c//2 and head-group g=c%2 (4 heads each).
Each core computes q/k/v projections for its heads, attention, and a partial
output projection over its heads' value dims; the host sums the two partial
outputs per batch and adds b_o.

Inputs (x, enc, all weights) are converted to bf16 on the host: the PE runs
bf16 at the same rate as fp32r while DMA traffic halves, which keeps the
front-of-kernel projections fed from a cold start. Attention internals
(kT/qT/vch/pT) stay fp32; the output is stored bf16 and accumulated in fp32
on the host.

Schedule notes:
- All phases share the 8 PSUM banks: projections use them as d-contraction
  accumulators, attention rotates scores through banks 0-3 with PV
  accumulators on 4-5 and softmax denominators in row 0 of banks 6-7, and
  the output projection rotates banks 0-2.
- b_v is folded into the V-projection PSUM->SBUF copies (broadcast tile),
  and b_q is added on the DVE, so the activation engine only does exps
  during attention (at 612ns/chunk it stays under the PE's 639ns/chunk).
- The last V-proj bank is deferred into the attention pipeline-fill bubble,
  and the V-proj copies are pipelined per-bank (jj_outer) so nothing gates
  the first scores.
- Both output projections are emitted after all attention work: the last
  head's normalize chain (recip -> broadcast -> mul) hides under the q2=0
  output projection instead of stalling the PE.
- A few 1-column warm-up matmuls absorb the PE p-state ramp while the first
  weight chunks stream in; a dummy exp preloads the Act Exp table.
"""

import math
import sys

import numpy as np

for _p in ("/opt/trn_rl_repo", "/root/.axon_site/_ro/trn_rl_repo"):
    if _p not in sys.path:
        sys.path.append(_p)

B = 4
LQ = 1024
LK = 2048
D = 1024
H = 8
DH = 128
P = 128
HPC = 4          # heads per core
OQ = HPC * DH    # 512 projected dims per core
NQ = 512         # matmul moving free dim
KC = D // P      # 8 contraction chunks for projections
LKC = LK // P    # 16 key chunks
N_CORES = 8

_BUILT = {}


def _build(masked):
    import concourse.bass as bass  # noqa: F401
    import concourse.tile as tile
    import concourse.mybir as mybir
    from concourse import bacc

    f32 = mybir.dt.float32
    f32r = mybir.dt.float32r
    bf16 = mybir.dt.bfloat16
    Exp = mybir.ActivationFunctionType.Exp

    nc = bacc.Bacc("TRN2", target_bir_lowering=False, debug=False,
                   num_devices=N_CORES)

    xT = nc.dram_tensor("xT", [D, LQ], bf16, kind="ExternalInput").ap()
    encT = nc.dram_tensor("encT", [D, LK], bf16, kind="ExternalInput").ap()
    wkT = nc.dram_tensor("wkT", [D, OQ], bf16, kind="ExternalInput").ap()
    # w_v and w_q interleaved per d-chunk: halves the descriptor count.
    wvqT = nc.dram_tensor("wvqT", [D, 2 * OQ], bf16, kind="ExternalInput").ap()
    woT = nc.dram_tensor("woT", [OQ, D], bf16, kind="ExternalInput").ap()
    bq_d = nc.dram_tensor("bq", [P, HPC], f32, kind="ExternalInput").ap()
    bk_d = nc.dram_tensor("bk", [P, HPC], f32, kind="ExternalInput").ap()
    bvb_d = nc.dram_tensor("bvb", [P, OQ], f32, kind="ExternalInput").ap()
    ones_d = nc.dram_tensor("ones", [P, 1], f32r, kind="ExternalInput").ap()
    if masked:
        maskT = nc.dram_tensor("maskT", [LK, LQ], f32, kind="ExternalInput").ap()
    out_d = nc.dram_tensor("out", [LQ, D], bf16, kind="ExternalOutput").ap()

    HLK = LK // 2  # 1024, one lk-half of the encoder

    with tile.TileContext(nc) as tc:
        with tc.tile_pool(name="persist", bufs=1) as persist:
            qT = [persist.tile([P, LQ], f32r, name=f"qT{h}") for h in range(HPC)]
            kT = [persist.tile([P, LK], f32r, name=f"kT{h}") for h in range(HPC)]
            vch = [persist.tile([P, OQ], f32r, name=f"v{j}") for j in range(LKC)]
            bq_sb = persist.tile([P, HPC], f32, name="bq")
            bk_sb = persist.tile([P, HPC], f32, name="bk")
            bv_sb = persist.tile([P, OQ], f32, name="bvb")
            ones_col = persist.tile([P, 1], f32r, name="ones")
            warm = persist.tile([1, HPC], f32, name="warm")
            wkc = [persist.tile([P, OQ], bf16, name=f"wk{d}") for d in range(KC)]
            wvqc = [persist.tile([P, 2 * OQ], bf16, name=f"wvq{d}")
                    for d in range(KC)]
            woch = [persist.tile([P, D], bf16, name=f"wo{h}")
                    for h in range(HPC)]
            valsT = [persist.tile([P, LQ], bf16, name=f"valsT{h}")
                     for h in range(HPC)]

            with (
                tc.tile_pool(name="acc", bufs=1, space="PSUM") as acc,
                tc.tile_pool(name="encp", bufs=1) as encp,
                tc.tile_pool(name="xh", bufs=1) as xhp,
            ):
                banks = [acc.tile([P, NQ], f32, name=f"bank{t}")
                         for t in range(8)]
                e0 = [encp.tile([P, HLK], bf16, name=f"e0_{d}")
                      for d in range(KC)]
                e1 = [encp.tile([P, HLK], bf16, name=f"e1_{d}")
                      for d in range(KC)]
                xf = [xhp.tile([P, LQ], bf16, name=f"x{d}")
                      for d in range(KC)]

                def kproj(e, lh, grp):
                    """kT for one lk-half: grp 0 -> banks 0-3, grp 1 -> 4-7."""
                    l2 = grp
                    for d in range(KC):
                        for h in range(HPC):
                            nc.tensor.matmul(
                                banks[grp * 4 + h][:],
                                wkc[d][:, h * DH:(h + 1) * DH],
                                e[d][:, l2 * NQ:(l2 + 1) * NQ],
                                start=(d == 0), stop=(d == KC - 1))
                    for h in range(HPC):
                        off = lh * HLK + l2 * NQ
                        nc.vector.tensor_scalar_add(
                            kT[h][:, off:off + NQ], banks[grp * 4 + h][:],
                            bk_sb[:, h:h + 1])

                def vproj(e, lh, grp, jj_outer=False, defer_last=False):
                    """v chunks j = lh*8 + grp*4 ... +4, with b_v folded in.

                    jj_outer finishes one bank at a time so the PSUM->SBUF
                    adds pipeline behind the matmuls instead of serializing
                    at the end (used for the last phase, which gates the
                    attention start). d-outer paces fresh DMA chunks.
                    defer_last holds back the final bank's work; it is
                    emitted later via vproj_deferred() to fill the attention
                    pipeline-fill bubble.
                    """
                    if jj_outer:
                        njj = 3 if defer_last else 4
                        for jj in range(njj):
                            jloc = grp * 4 + jj
                            for d in range(KC):
                                nc.tensor.matmul(
                                    banks[grp * 4 + jj][:],
                                    e[d][:, jloc * P:(jloc + 1) * P],
                                    wvqc[d][:, :OQ],
                                    start=(d == 0), stop=(d == KC - 1))
                            nc.vector.tensor_add(
                                vch[lh * 8 + jloc][:],
                                banks[grp * 4 + jj][:],
                                bv_sb[:])
                        return
                    for d in range(KC):
                        for jj in range(4):
                            jloc = grp * 4 + jj
                            nc.tensor.matmul(
                                banks[grp * 4 + jj][:],
                                e[d][:, jloc * P:(jloc + 1) * P],
                                wvqc[d][:, :OQ],
                                start=(d == 0), stop=(d == KC - 1))
                    for jj in range(4):
                        nc.vector.tensor_add(
                            vch[lh * 8 + grp * 4 + jj][:],
                            banks[grp * 4 + jj][:],
                            bv_sb[:])

                def qproj(grp):
                    """qT for query half q2=grp."""
                    q2 = grp
                    for d in range(KC):
                        for h in range(HPC):
                            nc.tensor.matmul(
                                banks[grp * 4 + h][:],
                                wvqc[d][:, OQ + h * DH:OQ + (h + 1) * DH],
                                xf[d][:, q2 * NQ:(q2 + 1) * NQ],
                                start=(d == 0), stop=(d == KC - 1))
                    for h in range(HPC):
                        nc.vector.tensor_scalar_add(
                            qT[h][:, q2 * NQ:(q2 + 1) * NQ],
                            banks[grp * 4 + h][:], bq_sb[:, h:h + 1])

                # DMA issue order is the descriptor service order: K-path
                # first, then biases, wv+wq, the second encoder half, x, wo.
                for d in range(KC):
                    nc.sync.dma_start(wkc[d][:], wkT[d * P:(d + 1) * P, :])
                    nc.sync.dma_start(e0[d][:], encT[d * P:(d + 1) * P, :HLK])
                    if d == 0:
                        # Warm-up: absorb the PE p-state ramp on 1-column
                        # matmuls while DMAs stream.
                        for _ in range(4):
                            nc.tensor.matmul(
                                banks[7][:1, :1],
                                wkc[0][:, :1], wkc[0][:, :1],
                                start=True, stop=True)
                for t, src in ((bk_sb, bk_d), (bv_sb, bvb_d), (bq_sb, bq_d),
                               (ones_col, ones_d)):
                    nc.sync.dma_start(t[:], src[:])
                # Preload the Exp table while the PE is busy projecting.
                nc.scalar.activation(warm[:], bq_sb[:1, :], Exp)
                for d in range(KC):
                    nc.sync.dma_start(wvqc[d][:], wvqT[d * P:(d + 1) * P, :])
                for d in range(KC):
                    nc.sync.dma_start(e1[d][:], encT[d * P:(d + 1) * P, HLK:])
                for d in range(KC):
                    nc.sync.dma_start(xf[d][:], xT[d * P:(d + 1) * P, :])
                for c in range(HPC):
                    nc.sync.dma_start(woch[c][:], woT[c * P:(c + 1) * P, :])

                # Q proj runs before the second V proj so the qT bias adds
                # (which gate the first attention scores) hide under V-proj
                # PE work.
                kproj(e0, 0, 0)
                kproj(e0, 0, 1)
                vproj(e0, 0, 0)
                vproj(e0, 0, 1)
                kproj(e1, 1, 0)
                kproj(e1, 1, 1)
                qproj(0)
                qproj(1)
                vproj(e1, 1, 0, jj_outer=True)
                vproj(e1, 1, 1, jj_outer=True, defer_last=True)

                def vproj_deferred():
                    """The held-back last V-proj bank (key chunk 15)."""
                    for d in range(KC):
                        nc.tensor.matmul(
                            banks[7][:],
                            e1[d][:, 7 * P:8 * P],
                            wvqc[d][:, :OQ],
                            start=(d == 0), stop=(d == KC - 1))
                    nc.vector.tensor_add(
                        vch[15][:], banks[7][:], bv_sb[:])

                # ---- Attention, directly on the projection PSUM banks:
                # scores rotate banks 0-3, PV accumulators alternate 4-5,
                # denominators live in row 0 of banks 6-7. The deferred
                # last V-proj bank (7) fills the exp-pipeline-fill bubble
                # at attention start.
                with (
                    tc.tile_pool(name="pTp", bufs=8) as pTp,
                    tc.tile_pool(name="smallp", bufs=2) as smallp,
                    tc.tile_pool(name="maskp", bufs=16 if masked else 1)
                        as maskp,
                    tc.tile_pool(name="osb", bufs=4) as osb,
                ):
                    for q2 in range(LQ // NQ):
                        if masked:
                            mch = []
                            for j in range(LKC):
                                mt = maskp.tile([P, NQ], f32, name=f"m{j}")
                                nc.sync.dma_start(
                                    mt[:],
                                    maskT[j * P:(j + 1) * P,
                                          q2 * NQ:(q2 + 1) * NQ])
                                mch.append(mt)
                        for h in range(HPC):
                            idx = q2 * HPC + h
                            ps_v = banks[4 + idx % 2]
                            ds = banks[6 + h % 2][:1, :]
                            for j in range(LKC):
                                ps_s = banks[j % 4]
                                nc.tensor.matmul(
                                    ps_s[:],
                                    kT[h][:, j * P:(j + 1) * P],
                                    qT[h][:, q2 * NQ:(q2 + 1) * NQ],
                                    start=True, stop=True)
                                if masked:
                                    nc.vector.tensor_add(
                                        ps_s[:], ps_s[:], mch[j][:])
                                pT = pTp.tile([P, NQ], f32r, name="pT")
                                nc.scalar.activation(pT[:], ps_s[:], Exp)
                                nc.tensor.matmul(
                                    ps_v[:],
                                    vch[j][:, h * DH:(h + 1) * DH],
                                    pT[:],
                                    start=(j == 0), stop=(j == LKC - 1))
                                nc.tensor.matmul(
                                    ds,
                                    ones_col[:],
                                    pT[:],
                                    start=(j == 0), stop=(j == LKC - 1))
                                if idx == 0 and j == 3:
                                    vproj_deferred()
                            recip = smallp.tile([1, NQ], f32, name="recip")
                            nc.vector.reciprocal(recip[:], ds)
                            bcast = smallp.tile([P, NQ], f32, name="bcast")
                            nc.gpsimd.partition_broadcast(bcast[:], recip[:])
                            nc.vector.tensor_mul(
                                valsT[h][:, q2 * NQ:(q2 + 1) * NQ],
                                ps_v[:], bcast[:])

                    # ---- Output projection, all of it after attention: the
                    # last head's normalize chain hides under the q2=0
                    # projection.
                    nout = 0
                    for q2 in range(LQ // NQ):
                        for lqc in range(q2 * 4, (q2 + 1) * 4):
                            for o2 in range(D // NQ):
                                po = banks[nout % 3]
                                nout += 1
                                for h in range(HPC):
                                    nc.tensor.matmul(
                                        po[:],
                                        valsT[h][:, lqc * P:(lqc + 1) * P],
                                        woch[h][:, o2 * NQ:(o2 + 1) * NQ],
                                        start=(h == 0), stop=(h == HPC - 1))
                                ot = osb.tile([P, NQ], bf16, name="ot")
                                nc.vector.tensor_copy(ot[:], po[:])
                                nc.sync.dma_start(
                                    out_d[lqc * P:(lqc + 1) * P,
                                          o2 * NQ:(o2 + 1) * NQ], ot[:])

    nc.compile()
    return nc


def _get_built(masked):
    if masked not in _BUILT:
        _BUILT[masked] = _build(masked)
    return _BUILT[masked]


def _shard_inputs(inputs, masked):
    import ml_dtypes

    bf16 = ml_dtypes.bfloat16

    x = np.asarray(inputs["mhca_input"], np.float32)
    enc = np.asarray(inputs["encoder_output"], np.float32)
    mask = np.asarray(inputs["cross_mask"], np.float32)
    W_kv = np.asarray(inputs["W_kv"], np.float32)
    b_kv = np.asarray(inputs["b_kv"], np.float32)
    W_q = np.asarray(inputs["W_q"], np.float32)
    b_q = np.asarray(inputs["b_q"], np.float32)
    W_o = np.asarray(inputs["W_o"], np.float32)

    scale = 1.0 / math.sqrt(DH)
    in_maps = []
    for c in range(N_CORES):
        b = c // 2
        g = c % 2
        heads = list(range(g * HPC, (g + 1) * HPC))
        sl = slice(g * OQ, (g + 1) * OQ)
        k_rows = np.concatenate(
            [W_kv[h * 2 * DH:h * 2 * DH + DH] for h in heads], 0)
        v_rows = np.concatenate(
            [W_kv[h * 2 * DH + DH:(h + 1) * 2 * DH] for h in heads], 0)
        bv_rows = np.concatenate(
            [b_kv[h * 2 * DH + DH:(h + 1) * 2 * DH] for h in heads], 0)
        wvq = np.concatenate([v_rows.T, (W_q[sl] * scale).T], axis=1)
        m = {
            "xT": np.ascontiguousarray(x[b].T).astype(bf16),
            "encT": np.ascontiguousarray(enc[b].T).astype(bf16),
            "wkT": np.ascontiguousarray(k_rows.T).astype(bf16),
            "wvqT": np.ascontiguousarray(wvq).astype(bf16),
            "woT": np.ascontiguousarray(W_o[:, sl].T).astype(bf16),
            "bq": np.ascontiguousarray((b_q[sl] * scale).reshape(HPC, DH).T),
            "bk": np.ascontiguousarray(
                np.stack([b_kv[h * 2 * DH:h * 2 * DH + DH] for h in heads], 1)),
            "bvb": np.ascontiguousarray(
                np.tile(bv_rows[None, :], (P, 1)).astype(np.float32)),
            "ones": np.ones((P, 1), np.float32),
        }
        if masked:
            m["maskT"] = np.ascontiguousarray(mask[b].T)
        in_maps.append(m)
    return in_maps


def kernel(mhca_input, encoder_output, cross_mask, W_kv, b_kv, W_q, b_q, W_o,
           b_o):
    from concourse.bass_utils import run_bass_kernel_spmd

    inputs = {
        "mhca_input": mhca_input, "encoder_output": encoder_output,
        "cross_mask": cross_mask, "W_kv": W_kv, "b_kv": b_kv, "W_q": W_q,
        "b_q": b_q, "W_o": W_o,
    }
    b_o = np.asarray(b_o, np.float32)
    masked = bool(np.any(np.asarray(cross_mask)))
    nc = _get_built(masked)
    in_maps = _shard_inputs(inputs, masked)

    res = run_bass_kernel_spmd(nc, in_maps, core_ids=list(range(N_CORES)))
    outs = [np.asarray(res.results[c]["out"], np.float32)
            for c in range(N_CORES)]
    full = np.stack([outs[2 * b] + outs[2 * b + 1] for b in range(B)], 0)
    return (full + b_o[None, None, :]).astype(np.float32)



# revision 9
# speedup vs baseline: 1.0343x; 1.0343x over previous
"""Multi-head cross-attention on 8 Trainium2 NeuronCores.

Problem shapes (hardcoded): B=4, Ld=1024, Le=2048, d_model=1024, 8 heads x 128.
Sharding: core c handles batch b=c//2 and head-group g=c%2 (4 heads each).
Each core computes q/k/v projections for its heads, attention, and a partial
output projection over its heads' value dims; the host sums the two partial
outputs per batch and adds b_o.

Inputs (x, enc, all weights) are converted to bf16 on the host: the PE runs
bf16 at the same rate as fp32r while DMA traffic halves, which keeps the
front-of-kernel projections fed from a cold start. Attention internals
(kT/qT/vch/pT) stay fp32; the output is stored bf16 and accumulated in fp32
on the host.

Schedule notes:
- All phases share the 8 PSUM banks: projections use them as d-contraction
  accumulators, attention rotates scores through banks 0-3 with PV
  accumulators on 4-5 and softmax denominators in row 0 of banks 6-7, and
  the output projection rotates banks 0-2.
- b_v is folded into the V-projection PSUM->SBUF copies (broadcast tile),
  and b_q is added on the DVE, so the activation engine only does exps
  during attention (at 612ns/chunk it stays under the PE's 639ns/chunk).
- The last V-proj bank is deferred into the attention pipeline-fill bubble,
  and the V-proj copies are pipelined per-bank (jj_outer) so nothing gates
  the first scores.
- Both output projections are emitted after all attention work: the last
  head's normalize chain (recip -> broadcast -> mul) hides under the q2=0
  output projection instead of stalling the PE.
- A few 1-column warm-up matmuls absorb the PE p-state ramp while the first
  weight chunks stream in; a dummy exp preloads the Act Exp table.
"""

import math
import sys

import numpy as np

for _p in ("/opt/trn_rl_repo", "/root/.axon_site/_ro/trn_rl_repo"):
    if _p not in sys.path:
        sys.path.append(_p)

B = 4
LQ = 1024
LK = 2048
D = 1024
H = 8
DH = 128
P = 128
HPC = 4          # heads per core
OQ = HPC * DH    # 512 projected dims per core
NQ = 512         # matmul moving free dim
KC = D // P      # 8 contraction chunks for projections
LKC = LK // P    # 16 key chunks
N_CORES = 8

_BUILT = {}


def _build(masked):
    import concourse.bass as bass  # noqa: F401
    import concourse.tile as tile
    import concourse.mybir as mybir
    from concourse import bacc

    f32 = mybir.dt.float32
    f32r = mybir.dt.float32r
    bf16 = mybir.dt.bfloat16
    Exp = mybir.ActivationFunctionType.Exp

    nc = bacc.Bacc("TRN2", target_bir_lowering=False, debug=False,
                   num_devices=N_CORES)

    xT = nc.dram_tensor("xT", [D, LQ], bf16, kind="ExternalInput").ap()
    encT = nc.dram_tensor("encT", [D, LK], bf16, kind="ExternalInput").ap()
    wkT = nc.dram_tensor("wkT", [D, OQ], bf16, kind="ExternalInput").ap()
    # w_v and w_q interleaved per d-chunk: halves the descriptor count.
    wvqT = nc.dram_tensor("wvqT", [D, 2 * OQ], bf16, kind="ExternalInput").ap()
    woT = nc.dram_tensor("woT", [OQ, D], bf16, kind="ExternalInput").ap()
    bq_d = nc.dram_tensor("bq", [P, HPC], f32, kind="ExternalInput").ap()
    bk_d = nc.dram_tensor("bk", [P, HPC], f32, kind="ExternalInput").ap()
    bvb_d = nc.dram_tensor("bvb", [P, OQ], f32, kind="ExternalInput").ap()
    ones_d = nc.dram_tensor("ones", [P, 1], bf16, kind="ExternalInput").ap()
    ident_d = nc.dram_tensor("identT", [P, P], f32, kind="ExternalInput").ap()
    if masked:
        maskT = nc.dram_tensor("maskT", [LK, LQ], f32, kind="ExternalInput").ap()
    out_d = nc.dram_tensor("out", [LQ, D], bf16, kind="ExternalOutput").ap()

    HLK = LK // 2  # 1024, one lk-half of the encoder

    with tile.TileContext(nc) as tc:
        with tc.tile_pool(name="persist", bufs=1) as persist:
            qT = [persist.tile([P, LQ], f32r, name=f"qT{h}") for h in range(HPC)]
            kT = [persist.tile([P, LK], f32r, name=f"kT{h}") for h in range(HPC)]
            vch = [persist.tile([P, OQ], bf16, name=f"v{j}") for j in range(LKC)]
            bq_sb = persist.tile([P, HPC], f32, name="bq")
            bk_sb = persist.tile([P, HPC], f32, name="bk")
            bv_sb = persist.tile([P, OQ], f32, name="bvb")
            ones_col = persist.tile([P, 1], bf16, name="ones")
            ident = persist.tile([P, P], f32, name="ident")
            warm = persist.tile([1, HPC], f32, name="warm")
            wkc = [persist.tile([P, OQ], bf16, name=f"wk{d}") for d in range(KC)]
            wvqc = [persist.tile([P, 2 * OQ], bf16, name=f"wvq{d}")
                    for d in range(KC)]
            woch = [persist.tile([P, D], bf16, name=f"wo{h}")
                    for h in range(HPC)]
            valsT = [persist.tile([P, LQ], bf16, name=f"valsT{h}")
                     for h in range(HPC)]

            with (
                tc.tile_pool(name="acc", bufs=1, space="PSUM") as acc,
                tc.tile_pool(name="encp", bufs=1) as encp,
                tc.tile_pool(name="xh", bufs=1) as xhp,
            ):
                banks = [acc.tile([P, NQ], f32, name=f"bank{t}")
                         for t in range(8)]
                e0 = [encp.tile([P, HLK], bf16, name=f"e0_{d}")
                      for d in range(KC)]
                e1 = [encp.tile([P, HLK], bf16, name=f"e1_{d}")
                      for d in range(KC)]
                xf = [xhp.tile([P, LQ], bf16, name=f"x{d}")
                      for d in range(KC)]

                def kproj(e, lh, grp):
                    """kT for one lk-half: grp 0 -> banks 0-3, grp 1 -> 4-7."""
                    l2 = grp
                    for d in range(KC):
                        for h in range(HPC):
                            nc.tensor.matmul(
                                banks[grp * 4 + h][:],
                                wkc[d][:, h * DH:(h + 1) * DH],
                                e[d][:, l2 * NQ:(l2 + 1) * NQ],
                                start=(d == 0), stop=(d == KC - 1))
                    for h in range(HPC):
                        off = lh * HLK + l2 * NQ
                        nc.vector.tensor_scalar_add(
                            kT[h][:, off:off + NQ], banks[grp * 4 + h][:],
                            bk_sb[:, h:h + 1])

                def vproj(e, lh, grp, jj_outer=False, defer_last=False):
                    """v chunks j = lh*8 + grp*4 ... +4, with b_v folded in.

                    jj_outer finishes one bank at a time so the PSUM->SBUF
                    adds pipeline behind the matmuls instead of serializing
                    at the end (used for the last phase, which gates the
                    attention start). d-outer paces fresh DMA chunks.
                    defer_last holds back the final bank's work; it is
                    emitted later via vproj_deferred() to fill the attention
                    pipeline-fill bubble.
                    """
                    if jj_outer:
                        njj = 3 if defer_last else 4
                        for jj in range(njj):
                            jloc = grp * 4 + jj
                            for d in range(KC):
                                nc.tensor.matmul(
                                    banks[grp * 4 + jj][:],
                                    e[d][:, jloc * P:(jloc + 1) * P],
                                    wvqc[d][:, :OQ],
                                    start=(d == 0), stop=(d == KC - 1))
                            nc.vector.tensor_add(
                                vch[lh * 8 + jloc][:],
                                banks[grp * 4 + jj][:],
                                bv_sb[:])
                        return
                    for d in range(KC):
                        for jj in range(4):
                            jloc = grp * 4 + jj
                            nc.tensor.matmul(
                                banks[grp * 4 + jj][:],
                                e[d][:, jloc * P:(jloc + 1) * P],
                                wvqc[d][:, :OQ],
                                start=(d == 0), stop=(d == KC - 1))
                    for jj in range(4):
                        nc.vector.tensor_add(
                            vch[lh * 8 + grp * 4 + jj][:],
                            banks[grp * 4 + jj][:],
                            bv_sb[:])

                def qproj(grp):
                    """qT for query half q2=grp."""
                    q2 = grp
                    for d in range(KC):
                        for h in range(HPC):
                            nc.tensor.matmul(
                                banks[grp * 4 + h][:],
                                wvqc[d][:, OQ + h * DH:OQ + (h + 1) * DH],
                                xf[d][:, q2 * NQ:(q2 + 1) * NQ],
                                start=(d == 0), stop=(d == KC - 1))
                    for h in range(HPC):
                        nc.vector.tensor_scalar_add(
                            qT[h][:, q2 * NQ:(q2 + 1) * NQ],
                            banks[grp * 4 + h][:], bq_sb[:, h:h + 1])

                # DMA issue order is the descriptor service order: K-path
                # first, then biases, wv+wq, the second encoder half, x, wo.
                for d in range(KC):
                    nc.sync.dma_start(wkc[d][:], wkT[d * P:(d + 1) * P, :])
                    nc.sync.dma_start(e0[d][:], encT[d * P:(d + 1) * P, :HLK])
                    if d == 0:
                        # Warm-up: absorb the PE p-state ramp on 1-column
                        # matmuls while DMAs stream.
                        for _ in range(4):
                            nc.tensor.matmul(
                                banks[7][:1, :1],
                                wkc[0][:, :1], wkc[0][:, :1],
                                start=True, stop=True)
                for t, src in ((bk_sb, bk_d), (bv_sb, bvb_d), (bq_sb, bq_d),
                               (ones_col, ones_d), (ident, ident_d)):
                    nc.sync.dma_start(t[:], src[:])
                # Preload the Exp table while the PE is busy projecting.
                nc.scalar.activation(warm[:], bq_sb[:1, :], Exp)
                for d in range(KC):
                    nc.sync.dma_start(wvqc[d][:], wvqT[d * P:(d + 1) * P, :])
                for d in range(KC):
                    nc.sync.dma_start(e1[d][:], encT[d * P:(d + 1) * P, HLK:])
                for d in range(KC):
                    nc.sync.dma_start(xf[d][:], xT[d * P:(d + 1) * P, :])
                for c in range(HPC):
                    nc.sync.dma_start(woch[c][:], woT[c * P:(c + 1) * P, :])

                # Q proj runs before the second V proj so the qT bias adds
                # (which gate the first attention scores) hide under V-proj
                # PE work.
                kproj(e0, 0, 0)
                kproj(e0, 0, 1)
                vproj(e0, 0, 0)
                vproj(e0, 0, 1)
                kproj(e1, 1, 0)
                kproj(e1, 1, 1)
                qproj(0)
                qproj(1)
                vproj(e1, 1, 0, jj_outer=True)
                vproj(e1, 1, 1, jj_outer=True, defer_last=True)

                def vproj_deferred():
                    """The held-back last V-proj bank (key chunk 15)."""
                    for d in range(KC):
                        nc.tensor.matmul(
                            banks[7][:],
                            e1[d][:, 7 * P:8 * P],
                            wvqc[d][:, :OQ],
                            start=(d == 0), stop=(d == KC - 1))
                    nc.vector.tensor_add(
                        vch[15][:], banks[7][:], bv_sb[:])

                # ---- Attention, directly on the projection PSUM banks:
                # scores rotate banks 0-3, PV accumulators alternate 4-5,
                # denominators live in row 0 of banks 6-7. The deferred
                # last V-proj bank (7) fills the exp-pipeline-fill bubble
                # at attention start.
                with (
                    tc.tile_pool(name="pTp", bufs=8) as pTp,
                    tc.tile_pool(name="smallp", bufs=2) as smallp,
                    tc.tile_pool(name="maskp", bufs=16 if masked else 1)
                        as maskp,
                    tc.tile_pool(name="osb", bufs=4) as osb,
                ):
                    for q2 in range(LQ // NQ):
                        if masked:
                            mch = []
                            for j in range(LKC):
                                mt = maskp.tile([P, NQ], f32, name=f"m{j}")
                                nc.sync.dma_start(
                                    mt[:],
                                    maskT[j * P:(j + 1) * P,
                                          q2 * NQ:(q2 + 1) * NQ])
                                mch.append(mt)
                        for h in range(HPC):
                            idx = q2 * HPC + h
                            ps_v = banks[4 + idx % 2]
                            dbank = banks[6 + idx % 2]
                            for j in range(LKC):
                                ps_s = banks[j % 4]
                                nc.tensor.matmul(
                                    ps_s[:],
                                    kT[h][:, j * P:(j + 1) * P],
                                    qT[h][:, q2 * NQ:(q2 + 1) * NQ],
                                    start=True, stop=True)
                                if masked:
                                    nc.vector.tensor_add(
                                        ps_s[:], ps_s[:], mch[j][:])
                                pT = pTp.tile([P, NQ], bf16, name="pT")
                                nc.scalar.activation(pT[:], ps_s[:], Exp)
                                nc.tensor.matmul(
                                    ps_v[:],
                                    vch[j][:, h * DH:(h + 1) * DH],
                                    pT[:],
                                    start=(j == 0), stop=(j == LKC - 1))
                                # Denominators via tall-skinny matmuls: pT is
                                # the stationary operand, a ones column moves,
                                # so each [128,1] partial-sum costs ~1 PE row
                                # instead of the 512 a [1,512] layout costs.
                                for s in range(4):
                                    nc.tensor.matmul(
                                        dbank[:, s:s + 1],
                                        pT[:, s * P:(s + 1) * P],
                                        ones_col[:],
                                        start=(j == 0 and s == 0),
                                        stop=(j == LKC - 1 and s == 3),
                                        skip_group_check=True)
                                if idx == 0 and j == 3:
                                    vproj_deferred()
                            # Normalize: d arrives q-on-partitions [128,4];
                            # reciprocal, then PE-transpose each column into
                            # a [1,512] row (in the opposite-parity d bank)
                            # and broadcast it across partitions to line up
                            # with the q-on-free vals layout.
                            bankT = banks[6 + (idx + 1) % 2]
                            rsb = smallp.tile([P, 4], f32, name="rsb")
                            nc.vector.reciprocal(rsb[:], dbank[:, 0:4])
                            for s in range(4):
                                nc.tensor.transpose(
                                    bankT[0:1, s * P:(s + 1) * P],
                                    rsb[:, s:s + 1], ident[:])
                            rrow = smallp.tile([1, NQ], f32, name="rrow")
                            nc.vector.tensor_copy(rrow[:], bankT[0:1, :])
                            bcast = smallp.tile([P, NQ], f32, name="bcast")
                            nc.gpsimd.partition_broadcast(bcast[:], rrow[:])
                            nc.vector.tensor_mul(
                                valsT[h][:, q2 * NQ:(q2 + 1) * NQ],
                                ps_v[:], bcast[:])

                    # ---- Output projection, all of it after attention: the
                    # last head's normalize chain hides under the q2=0
                    # projection.
                    nout = 0
                    for q2 in range(LQ // NQ):
                        for lqc in range(q2 * 4, (q2 + 1) * 4):
                            for o2 in range(D // NQ):
                                po = banks[nout % 3]
                                nout += 1
                                for h in range(HPC):
                                    nc.tensor.matmul(
                                        po[:],
                                        valsT[h][:, lqc * P:(lqc + 1) * P],
                                        woch[h][:, o2 * NQ:(o2 + 1) * NQ],
                                        start=(h == 0), stop=(h == HPC - 1))
                                ot = osb.tile([P, NQ], bf16, name="ot")
                                nc.vector.tensor_copy(ot[:], po[:])
                                nc.sync.dma_start(
                                    out_d[lqc * P:(lqc + 1) * P,
                                          o2 * NQ:(o2 + 1) * NQ], ot[:])

    nc.compile()
    return nc


def _get_built(masked):
    if masked not in _BUILT:
        _BUILT[masked] = _build(masked)
    return _BUILT[masked]


def _shard_inputs(inputs, masked):
    import ml_dtypes

    bf16 = ml_dtypes.bfloat16

    x = np.asarray(inputs["mhca_input"], np.float32)
    enc = np.asarray(inputs["encoder_output"], np.float32)
    mask = np.asarray(inputs["cross_mask"], np.float32)
    W_kv = np.asarray(inputs["W_kv"], np.float32)
    b_kv = np.asarray(inputs["b_kv"], np.float32)
    W_q = np.asarray(inputs["W_q"], np.float32)
    b_q = np.asarray(inputs["b_q"], np.float32)
    W_o = np.asarray(inputs["W_o"], np.float32)

    scale = 1.0 / math.sqrt(DH)
    in_maps = []
    for c in range(N_CORES):
        b = c // 2
        g = c % 2
        heads = list(range(g * HPC, (g + 1) * HPC))
        sl = slice(g * OQ, (g + 1) * OQ)
        k_rows = np.concatenate(
            [W_kv[h * 2 * DH:h * 2 * DH + DH] for h in heads], 0)
        v_rows = np.concatenate(
            [W_kv[h * 2 * DH + DH:(h + 1) * 2 * DH] for h in heads], 0)
        bv_rows = np.concatenate(
            [b_kv[h * 2 * DH + DH:(h + 1) * 2 * DH] for h in heads], 0)
        wvq = np.concatenate([v_rows.T, (W_q[sl] * scale).T], axis=1)
        m = {
            "xT": np.ascontiguousarray(x[b].T).astype(bf16),
            "encT": np.ascontiguousarray(enc[b].T).astype(bf16),
            "wkT": np.ascontiguousarray(k_rows.T).astype(bf16),
            "wvqT": np.ascontiguousarray(wvq).astype(bf16),
            "woT": np.ascontiguousarray(W_o[:, sl].T).astype(bf16),
            "bq": np.ascontiguousarray((b_q[sl] * scale).reshape(HPC, DH).T),
            "bk": np.ascontiguousarray(
                np.stack([b_kv[h * 2 * DH:h * 2 * DH + DH] for h in heads], 1)),
            "bvb": np.ascontiguousarray(
                np.tile(bv_rows[None, :], (P, 1)).astype(np.float32)),
            "ones": np.ones((P, 1), bf16),
            "identT": np.eye(P, dtype=np.float32),
        }
        if masked:
            m["maskT"] = np.ascontiguousarray(mask[b].T)
        in_maps.append(m)
    return in_maps


def kernel(mhca_input, encoder_output, cross_mask, W_kv, b_kv, W_q, b_q, W_o,
           b_o):
    from concourse.bass_utils import run_bass_kernel_spmd

    inputs = {
        "mhca_input": mhca_input, "encoder_output": encoder_output,
        "cross_mask": cross_mask, "W_kv": W_kv, "b_kv": b_kv, "W_q": W_q,
        "b_q": b_q, "W_o": W_o,
    }
    b_o = np.asarray(b_o, np.float32)
    masked = bool(np.any(np.asarray(cross_mask)))
    nc = _get_built(masked)
    in_maps = _shard_inputs(inputs, masked)

    res = run_bass_kernel_spmd(nc, in_maps, core_ids=list(range(N_CORES)))
    outs = [np.asarray(res.results[c]["out"], np.float32)
            for c in range(N_CORES)]
    full = np.stack([outs[2 * b] + outs[2 * b + 1] for b in range(B)], 0)
    return (full + b_o[None, None, :]).astype(np.float32)

